# revision 43
# baseline (speedup 1.0000x reference)
"""Modulated deformable conv (warp-norm softmax weights) on 8 TRN2 NeuronCores.

Sharding: 8 cores = (batch 4) x (image half 2). Each core gets a 96-row band
of x as a prebuilt row-pair quad table (host layout prep), host-computed
wrapped gather indices + bilinear corner weights, and computes
out[64, 80, 160] f32.

Device pipeline per 256-px block (50 blocks, 2 chunks each):
  1. dma_gather from the DRAM quad table (one 512B read = the 2x2 corner
     quad for a (pixel, tap) sample; pixel-major output). Gathers rotate
     over 3 SWDGE queues: each queue's descriptor generation runs on its
     own Q7 core pair, so 3 gathers generate concurrently. 3 queues (not
     4) keeps aggregate desc-gen just *below* the DMA engines' 512B-read
     drain rate -- at 4 queues generation overruns the drain, the
     descriptor rings fill, and completion convoys stall the pipeline.
  2. DVE: Gw = G * wq (free-dim-broadcast corner weights, mask and wy
     folded in); x-fold Gy = Gw[xj=0]+Gw[xj=1]; y-fold Gz = Gy[y0]+Gy[y1].
  3. PE: transpose Gz tap-PAIR slices to (2 taps x 64ch)-major (4x
     [128,128] + 1x [128,64] per chunk); GEMM with tap-paired softmaxed
     weights (contraction 128 = 2 taps x 64ch, 4 matmuls + 1 K=64 matmul
     per group) accumulating in PSUM; ACT evacuates; per-block DMA out.

Host prep (numpy): quad table (transpose+pair-duplicate of the x band,
f32->bf16), softmax weights in tap-paired stationary layout, bilinear
corner index/weight computation with the reference's independently-
clipped, validity-zeroed corner semantics via slot-remapping, emitted
directly in the gather's 16-partition-wrapped index layout.
"""
import os
import sys

sys.path.insert(0, "/opt/trn_rl_repo")

import numpy as np
import ml_dtypes

import concourse.bass as bass
import concourse.bacc as bacc
import concourse.mybir as mybir
from concourse.tile import TileContext
from concourse.masks import make_identity
from concourse.bass_utils import run_bass_kernel_spmd

bf16 = ml_dtypes.bfloat16
f32 = mybir.dt.float32
bft = mybir.dt.bfloat16
i16 = mybir.dt.int16

H = W = 160
CIN = OC = 64
K = 3
K2 = 9
BAND = 96
OUT_ROWS = 80
NP = OUT_ROWS * W          # 12800
NCHUNK = NP // 128         # 100
NIDX = NP * K2             # 115200
N_TAB = BAND * W           # 15360 table rows (only 0..15199 addressable)
BLOCKS = [2] * 50  # chunks per gather block (sum = NCHUNK)

_CACHE = {}
LAST_RESULTS = {}


def _build_program():
    nc = bacc.Bacc("TRN2", num_devices=8, num_swdge_queues=3)

    tab_in = nc.dram_tensor("tab", [N_TAB, 2 * CIN], bft, kind="ExternalInput")
    idxw_in = nc.dram_tensor("idxw", [128, NIDX // 16], i16, kind="ExternalInput")
    wq2_in = nc.dram_tensor("wq2", [128, NCHUNK * 72], bft, kind="ExternalInput")
    wsm2_in = nc.dram_tensor("wsm2", [128, 5 * OC], bft, kind="ExternalInput")
    out_t = nc.dram_tensor("out", [OC, NP], f32, kind="ExternalOutput")

    tab_ap = bass.AP(tab_in, 0, [[2 * CIN, N_TAB - W], [1, 4 * CIN]])

    b0c = BLOCKS[0] * K2 * 8   # block-0 idx columns

    with TileContext(nc) as tc:
        with tc.tile_pool(name="const", bufs=1) as cpool:
            idx0 = cpool.tile([128, b0c], i16)
            nc.scalar.dma_start(idx0[:], idxw_in[:, :b0c])
            wq2 = cpool.tile([128, NCHUNK, 36, 2], bft)
            nc.sync.dma_start(wq2[:], wq2_in[:])
            idxr = cpool.tile([128, NIDX // 16 - b0c], i16)
            nc.sync.dma_start(idxr[:], idxw_in[:, b0c:])
            wsm2 = cpool.tile([128, 5, OC], bft)
            nc.sync.dma_start(wsm2[:], wsm2_in[:])
            ident = cpool.tile([128, 128], bft)
            make_identity(nc, ident[:])

            with tc.tile_pool(name="gth", bufs=12) as gp, \
                 tc.tile_pool(name="gwp", bufs=3) as gwp, \
                 tc.tile_pool(name="gyp", bufs=4) as gyp, \
                 tc.tile_pool(name="gzp", bufs=6) as gzp, \
                 tc.tile_pool(name="gyt", bufs=2) as yp, \
                 tc.tile_pool(name="osb", bufs=2) as op, \
                 tc.tile_pool(name="trp", bufs=3, space="PSUM") as prp, \
                 tc.tile_pool(name="acp", bufs=2, space="PSUM") as acp:
                ch0 = 0
                for blk, bpc in enumerate(BLOCKS):
                    bidx = bpc * K2 * 128
                    g = gp.tile([128, bpc * K2, 4 * CIN], bft, name="g")
                    if blk == 0:
                        idxs_ap = idx0[:]
                    else:
                        s0 = ch0 * K2 * 8 - b0c
                        idxs_ap = idxr[:, s0:s0 + bidx // 16]
                    nc.gpsimd.dma_gather(
                        out_ap=g[:],
                        in_ap=tab_ap,
                        idxs_ap=idxs_ap,
                        num_idxs=bidx,
                        num_idxs_reg=bidx,
                        elem_size=4 * CIN,
                        elem_step=2 * CIN,
                        single_packet=False,
                        queue_num=blk % 3,
                    )
                    gys = []
                    for c in range(bpc):
                        ch = ch0 + c
                        gw = gwp.tile([128, K2 * 4 * CIN], bft, tag="gw")
                        g_ap = g[:]
                        in0 = bass.AP(g_ap.tensor,
                                      g_ap.offset + c * K2 * 4 * CIN,
                                      [g_ap.ap[0], [CIN, 4 * K2],
                                       [2, CIN // 2], [1, 2]])
                        w_ap = wq2[:]
                        in1 = bass.AP(w_ap.tensor, w_ap.offset + ch * 72,
                                      [w_ap.ap[0], [2, 4 * K2],
                                       [0, CIN // 2], [1, 2]])
                        o_ap2 = gw[:]
                        o4 = bass.AP(o_ap2.tensor, o_ap2.offset,
                                     [o_ap2.ap[0], [CIN, 4 * K2],
                                      [2, CIN // 2], [1, 2]])
                        nc.vector.tensor_tensor(out=o4, in0=in0, in1=in1,
                                                op=mybir.AluOpType.mult)
                        gy = gyp.tile([128, K2 * 2 * CIN], bft, tag="gy")
                        a0 = bass.AP(o_ap2.tensor, o_ap2.offset,
                                     [o_ap2.ap[0], [4 * CIN, K2],
                                      [1, 2 * CIN]])
                        a1 = bass.AP(o_ap2.tensor, o_ap2.offset + 2 * CIN,
                                     [o_ap2.ap[0], [4 * CIN, K2],
                                      [1, 2 * CIN]])
                        nc.vector.tensor_tensor(out=gy[:], in0=a0, in1=a1,
                                                op=mybir.AluOpType.add)
                        # y-corner fold on DVE: gz[p, k, c] = gy[..y0..] + gy[..y1..]
                        gz = gzp.tile([128, K2 * CIN], bft, tag="gz")
                        gy_ap = gy[:]
                        c0 = bass.AP(gy_ap.tensor, gy_ap.offset,
                                     [gy_ap.ap[0], [2 * CIN, K2], [1, CIN]])
                        c1 = bass.AP(gy_ap.tensor, gy_ap.offset + CIN,
                                     [gy_ap.ap[0], [2 * CIN, K2], [1, CIN]])
                        nc.vector.tensor_tensor(out=gz[:], in0=c0, in1=c1,
                                                op=mybir.AluOpType.add)
                        gys.append(gz)
                    # transpose tap-pair slices: 4x [128,128] + 1x [128,64]
                    gyt = yp.tile([128, 5, bpc * 128], bft, name="gyt")
                    for s in range(4):
                        pst = prp.tile([128, bpc * 128], bft, name="pst")
                        for c in range(bpc):
                            nc.tensor.transpose(
                                pst[:, c * 128:(c + 1) * 128],
                                gys[c][:, s * 128:(s + 1) * 128], ident[:])
                        nc.scalar.copy(gyt[:, s, :], pst[:])
                    pst4 = prp.tile([64, bpc * 128], bft, name="pst4")
                    for c in range(bpc):
                        nc.tensor.transpose(
                            pst4[:, c * 128:(c + 1) * 128],
                            gys[c][:, 512:576], ident[:])
                    nc.scalar.copy(gyt[:64, 4, :], pst4[:])
                    osb = op.tile([OC, bpc * 128], f32, name="osb")
                    for g0 in range(0, bpc * 128, 512):
                        gw_ = min(512, bpc * 128 - g0)
                        acc = acp.tile([OC, gw_], f32, name="acc")
                        for s in range(4):
                            nc.tensor.matmul(
                                acc[:], wsm2[:, s, :],
                                gyt[:, s, g0:g0 + gw_],
                                start=(s == 0), stop=False)
                        nc.tensor.matmul(
                            acc[:], wsm2[:64, 4, :],
                            gyt[:64, 4, g0:g0 + gw_],
                            start=False, stop=True)
                        nc.scalar.copy(osb[:, g0:g0 + gw_], acc[:])
                    eng = nc.sync if blk % 2 == 0 else nc.scalar
                    eng.dma_start(
                        bass.AP(out_t, ch0 * 128,
                                [[NP, OC], [1, bpc * 128]]),
                        osb[:])
                    ch0 += bpc

    nc.compile()
    return nc


def _host_inputs(x, offset, mask, weight):
    B = x.shape[0]
    w = np.exp(weight - weight.max(axis=2, keepdims=True))
    wsm = (w / w.sum(axis=2, keepdims=True)).astype(np.float32)
    wT = np.transpose(wsm, (2, 1, 0))                        # [k, c, oc]
    # tap-paired stationary: row r of pair j -> tap 2j + r//64, chan r%64
    wsm2 = np.zeros((128, 5, OC), np.float32)
    wsm2[:64, :4] = np.transpose(wT[0:8:2], (1, 0, 2))
    wsm2[64:, :4] = np.transpose(wT[1:8:2], (1, 0, 2))
    wsm2[:64, 4] = wT[8]
    wsm2 = np.ascontiguousarray(
        wsm2.reshape(128, 5 * OC).astype(bf16))

    kh = (np.arange(K2) // K).astype(np.float32)             # [9]
    kw = (np.arange(K2) % K).astype(np.float32)

    in_maps, meta = [], []
    for b in range(B):
        for h in range(2):
            lo = 0 if h == 0 else H - BAND
            out_lo = 0 if h == 0 else H - OUT_ROWS

            # --- quad table: entry e = [xT[e] | xT[e+160]] bf16 ---
            xb = x[b, :, lo:lo + BAND, :].reshape(CIN, N_TAB)
            xT = np.ascontiguousarray(xb.T).astype(bf16)     # [15360, 64]
            tab = np.zeros((N_TAB, 2 * CIN), bf16)
            tab[:N_TAB - W, :CIN] = xT[:N_TAB - W]
            tab[:N_TAB - W, CIN:] = xT[W:]

            # --- per-pixel sample positions (f32, mirrors reference) ---
            osl = offset[b, :, out_lo:out_lo + OUT_ROWS, :].reshape(18, NP)
            msl = mask[b, :, out_lo:out_lo + OUT_ROWS, :].reshape(K2, NP)
            rr = (out_lo + np.arange(OUT_ROWS, dtype=np.float32))[:, None] \
                .repeat(W, 1).reshape(NP)
            cc = np.arange(W, dtype=np.float32)[None, :] \
                .repeat(OUT_ROWS, 0).reshape(NP)
            py = (osl[0::2] + (rr[None, :] - 1.0 + kh[:, None])) \
                .astype(np.float32)                          # [9, NP]
            px = (osl[1::2] + (cc[None, :] - 1.0 + kw[:, None])) \
                .astype(np.float32)

            y0 = np.floor(py)
            x0 = np.floor(px)
            fy = py - y0
            fx = px - x0
            ly = y0 - np.float32(lo)
            ecy = np.clip(ly, 0.0, float(BAND - 2))
            tey = ly - ecy
            ecx = np.clip(x0, 0.0, float(W - 2))
            tex = x0 - ecx
            m = msl
            wyA = m * (np.where(tey == 0.0, 1.0 - fy, 0.0)
                       + np.where(tey == -1.0, fy, 0.0))
            wyB = m * (np.where(tey == 0.0, fy, 0.0)
                       + np.where(tey == 1.0, 1.0 - fy, 0.0))
            wxA = (np.where(tex == 0.0, 1.0 - fx, 0.0)
                   + np.where(tex == -1.0, fx, 0.0))
            wxB = (np.where(tex == 0.0, fx, 0.0)
                   + np.where(tex == 1.0, 1.0 - fx, 0.0))

            idx = (ecy * W + ecx).astype(np.int16)           # [9, NP]

            # wq[k, xj, yj, px] corner weights, mask folded into wy
            wq = np.empty((K2, 2, 2, NP), np.float32)
            wq[:, 0, 0] = wxA * wyA
            wq[:, 0, 1] = wxA * wyB
            wq[:, 1, 0] = wxB * wyA
            wq[:, 1, 1] = wxB * wyB

            # device wq2 layout: [128 p, c, (k,xj,yj), dup2] bf16
            wq_p = wq.reshape(K2, 4, NCHUNK, 128)            # px = c*128+p
            wq2 = np.transpose(wq_p, (3, 2, 0, 1))           # [p, c, k, j]
            wq2 = np.repeat(
                wq2.reshape(128, NCHUNK * 36), 2, axis=1).astype(bf16)

            # idx_wrap[p, (c*9+k)*8+dp] = idx[k, (c*128 + dp*16 + p%16)]
            idx_a = idx.reshape(K2, NCHUNK, 8, 16)           # [k, c, dp, q]
            wrap16 = np.transpose(idx_a, (3, 1, 0, 2))       # [q, c, k, dp]
            idx_wrap = np.tile(
                np.ascontiguousarray(wrap16.reshape(16, NIDX // 16)), (8, 1))

            in_maps.append({
                "tab": tab,
                "idxw": idx_wrap,
                "wq2": np.ascontiguousarray(wq2),
                "wsm2": wsm2,
            })
            meta.append((b, out_lo))
    return in_maps, meta


def kernel(x, offset, mask, weight):
    x = np.asarray(x, dtype=np.float32)
    offset = np.asarray(offset, dtype=np.float32)
    mask = np.asarray(mask, dtype=np.float32)
    weight = np.asarray(weight, dtype=np.float32)

    if "nc" not in _CACHE:
        _CACHE["nc"] = _build_program()
    nc = _CACHE["nc"]

    in_maps, meta = _host_inputs(x, offset, mask, weight)
    trace = os.environ.get("DEFORM_TRACE", "0") == "1"
    res = run_bass_kernel_spmd(nc, in_maps, core_ids=list(range(8)),
                               trace=trace)
    LAST_RESULTS["exec_time_ns"] = res.exec_time_ns
    LAST_RESULTS["mean_exec_time_ns"] = res.mean_exec_time_ns

    B = x.shape[0]
    out = np.zeros((B, OC, H, W), np.float32)
    for i, (b, out_lo) in enumerate(meta):
        out[b, :, out_lo:out_lo + OUT_ROWS, :] = \
            res.results[i]["out"].reshape(OC, OUT_ROWS, W)
    return out



# revision 45
# speedup vs baseline: 1.0423x; 1.0423x over previous
"""Modulated deformable conv (warp-norm softmax weights) on 8 TRN2 NeuronCores.

Sharding: 8 cores = (batch 4) x (image half 2). Each core gets a 96-row band
of x as a prebuilt row-pair quad table (host layout prep), host-computed
wrapped gather indices + bilinear corner weights, and computes
out[64, 80, 160] f32.

Device pipeline per 256-px block (50 blocks, 2 chunks each):
  1. dma_gather from the DRAM quad table (one 512B read = the 2x2 corner
     quad for a (pixel, tap) sample; pixel-major output). Gathers rotate
     over 3 SWDGE queues: each queue's descriptor generation runs on its
     own Q7 core pair, so 3 gathers generate concurrently. 3 queues (not
     4) keeps aggregate desc-gen just *below* the DMA engines' 512B-read
     drain rate -- at 4 queues generation overruns the drain, the
     descriptor rings fill, and completion convoys stall the pipeline.
  2. DVE: Gw = G * wq (free-dim-broadcast corner weights, mask and wy
     folded in); x-fold Gy = Gw[xj=0]+Gw[xj=1]; y-fold Gz = Gy[y0]+Gy[y1].
  3. PE: transpose Gz tap-PAIR slices to (2 taps x 64ch)-major (4x
     [128,128] + 1x [128,64] per chunk); GEMM with tap-paired softmaxed
     weights (contraction 128 = 2 taps x 64ch, 4 matmuls + 1 K=64 matmul
     per group) accumulating in PSUM; ACT evacuates; per-block DMA out.

Host prep (numpy): quad table (transpose+pair-duplicate of the x band,
f32->bf16), softmax weights in tap-paired stationary layout, bilinear
corner index/weight computation with the reference's independently-
clipped, validity-zeroed corner semantics via slot-remapping, emitted
directly in the gather's 16-partition-wrapped index layout.
"""
import os
import sys

sys.path.insert(0, "/opt/trn_rl_repo")

import numpy as np
import ml_dtypes

import concourse.bass as bass
import concourse.bacc as bacc
import concourse.mybir as mybir
from concourse.tile import TileContext
from concourse.masks import make_identity
from concourse.bass_utils import run_bass_kernel_spmd

bf16 = ml_dtypes.bfloat16
f32 = mybir.dt.float32
bft = mybir.dt.bfloat16
i16 = mybir.dt.int16

H = W = 160
CIN = OC = 64
K = 3
K2 = 9
BAND = 96
OUT_ROWS = 80
NP = OUT_ROWS * W          # 12800
NCHUNK = NP // 128         # 100
NIDX = NP * K2             # 115200
N_TAB = BAND * W           # 15360 table rows (only 0..15199 addressable)
BLOCKS = [2] * 50  # chunks per gather block (sum = NCHUNK)

_CACHE = {}
LAST_RESULTS = {}


def _build_program():
    nc = bacc.Bacc("TRN2", num_devices=8, num_swdge_queues=4)

    tab_in = nc.dram_tensor("tab", [N_TAB, 2 * CIN], bft, kind="ExternalInput")
    idxw_in = nc.dram_tensor("idxw", [128, NIDX // 16], i16, kind="ExternalInput")
    wq2_in = nc.dram_tensor("wq2", [128, NCHUNK * 72], bft, kind="ExternalInput")
    wsm2_in = nc.dram_tensor("wsm2", [128, 5 * OC], bft, kind="ExternalInput")
    out_t = nc.dram_tensor("out", [OC, NP], f32, kind="ExternalOutput")

    tab_ap = bass.AP(tab_in, 0, [[2 * CIN, N_TAB - W], [1, 4 * CIN]])

    b0c = BLOCKS[0] * K2 * 8   # block-0 idx columns

    with TileContext(nc) as tc:
        with tc.tile_pool(name="const", bufs=1) as cpool:
            idx0 = cpool.tile([128, b0c], i16)
            nc.scalar.dma_start(idx0[:], idxw_in[:, :b0c])
            wq2 = cpool.tile([128, NCHUNK, 36, 2], bft)
            nc.sync.dma_start(wq2[:], wq2_in[:])
            idxr = cpool.tile([128, NIDX // 16 - b0c], i16)
            nc.sync.dma_start(idxr[:], idxw_in[:, b0c:])
            wsm2 = cpool.tile([128, 5, OC], bft)
            nc.sync.dma_start(wsm2[:], wsm2_in[:])
            ident = cpool.tile([128, 128], bft)
            make_identity(nc, ident[:])

            with tc.tile_pool(name="gth", bufs=12) as gp, \
                 tc.tile_pool(name="gwp", bufs=3) as gwp, \
                 tc.tile_pool(name="gyp", bufs=4) as gyp, \
                 tc.tile_pool(name="gzp", bufs=6) as gzp, \
                 tc.tile_pool(name="gyt", bufs=2) as yp, \
                 tc.tile_pool(name="osb", bufs=2) as op, \
                 tc.tile_pool(name="trp", bufs=3, space="PSUM") as prp, \
                 tc.tile_pool(name="acp", bufs=2, space="PSUM") as acp:
                ch0 = 0
                for blk, bpc in enumerate(BLOCKS):
                    bidx = bpc * K2 * 128
                    g = gp.tile([128, bpc * K2, 4 * CIN], bft, name="g")
                    if blk == 0:
                        idxs_ap = idx0[:]
                    else:
                        s0 = ch0 * K2 * 8 - b0c
                        idxs_ap = idxr[:, s0:s0 + bidx // 16]
                    nc.gpsimd.dma_gather(
                        out_ap=g[:],
                        in_ap=tab_ap,
                        idxs_ap=idxs_ap,
                        num_idxs=bidx,
                        num_idxs_reg=bidx,
                        elem_size=4 * CIN,
                        elem_step=2 * CIN,
                        single_packet=False,
                        # 13-cycle (4,3,3,3) queue loads: aggregate desc-gen
                        # ~5.63us/block, above the DMA drain rate (no convoy)
                        # but 8% faster than uniform 3-queue rotation
                        queue_num=(blk % 13) % 4,
                    )
                    gys = []
                    for c in range(bpc):
                        ch = ch0 + c
                        gw = gwp.tile([128, K2 * 4 * CIN], bft, tag="gw")
                        g_ap = g[:]
                        in0 = bass.AP(g_ap.tensor,
                                      g_ap.offset + c * K2 * 4 * CIN,
                                      [g_ap.ap[0], [CIN, 4 * K2],
                                       [2, CIN // 2], [1, 2]])
                        w_ap = wq2[:]
                        in1 = bass.AP(w_ap.tensor, w_ap.offset + ch * 72,
                                      [w_ap.ap[0], [2, 4 * K2],
                                       [0, CIN // 2], [1, 2]])
                        o_ap2 = gw[:]
                        o4 = bass.AP(o_ap2.tensor, o_ap2.offset,
                                     [o_ap2.ap[0], [CIN, 4 * K2],
                                      [2, CIN // 2], [1, 2]])
                        nc.vector.tensor_tensor(out=o4, in0=in0, in1=in1,
                                                op=mybir.AluOpType.mult)
                        gy = gyp.tile([128, K2 * 2 * CIN], bft, tag="gy")
                        a0 = bass.AP(o_ap2.tensor, o_ap2.offset,
                                     [o_ap2.ap[0], [4 * CIN, K2],
                                      [1, 2 * CIN]])
                        a1 = bass.AP(o_ap2.tensor, o_ap2.offset + 2 * CIN,
                                     [o_ap2.ap[0], [4 * CIN, K2],
                                      [1, 2 * CIN]])
                        nc.vector.tensor_tensor(out=gy[:], in0=a0, in1=a1,
                                                op=mybir.AluOpType.add)
                        # y-corner fold on DVE: gz[p, k, c] = gy[..y0..] + gy[..y1..]
                        gz = gzp.tile([128, K2 * CIN], bft, tag="gz")
                        gy_ap = gy[:]
                        c0 = bass.AP(gy_ap.tensor, gy_ap.offset,
                                     [gy_ap.ap[0], [2 * CIN, K2], [1, CIN]])
                        c1 = bass.AP(gy_ap.tensor, gy_ap.offset + CIN,
                                     [gy_ap.ap[0], [2 * CIN, K2], [1, CIN]])
                        nc.vector.tensor_tensor(out=gz[:], in0=c0, in1=c1,
                                                op=mybir.AluOpType.add)
                        gys.append(gz)
                    # transpose tap-pair slices: 4x [128,128] + 1x [128,64]
                    gyt = yp.tile([128, 5, bpc * 128], bft, name="gyt")
                    for s in range(4):
                        pst = prp.tile([128, bpc * 128], bft, name="pst")
                        for c in range(bpc):
                            nc.tensor.transpose(
                                pst[:, c * 128:(c + 1) * 128],
                                gys[c][:, s * 128:(s + 1) * 128], ident[:])
                        nc.scalar.copy(gyt[:, s, :], pst[:])
                    pst4 = prp.tile([64, bpc * 128], bft, name="pst4")
                    for c in range(bpc):
                        nc.tensor.transpose(
                            pst4[:, c * 128:(c + 1) * 128],
                            gys[c][:, 512:576], ident[:])
                    nc.scalar.copy(gyt[:64, 4, :], pst4[:])
                    osb = op.tile([OC, bpc * 128], f32, name="osb")
                    for g0 in range(0, bpc * 128, 512):
                        gw_ = min(512, bpc * 128 - g0)
                        acc = acp.tile([OC, gw_], f32, name="acc")
                        for s in range(4):
                            nc.tensor.matmul(
                                acc[:], wsm2[:, s, :],
                                gyt[:, s, g0:g0 + gw_],
                                start=(s == 0), stop=False)
                        nc.tensor.matmul(
                            acc[:], wsm2[:64, 4, :],
                            gyt[:64, 4, g0:g0 + gw_],
                            start=False, stop=True)
                        nc.scalar.copy(osb[:, g0:g0 + gw_], acc[:])
                    eng = nc.sync if blk % 2 == 0 else nc.scalar
                    eng.dma_start(
                        bass.AP(out_t, ch0 * 128,
                                [[NP, OC], [1, bpc * 128]]),
                        osb[:])
                    ch0 += bpc

    nc.compile()
    return nc


def _host_inputs(x, offset, mask, weight):
    B = x.shape[0]
    w = np.exp(weight - weight.max(axis=2, keepdims=True))
    wsm = (w / w.sum(axis=2, keepdims=True)).astype(np.float32)
    wT = np.transpose(wsm, (2, 1, 0))                        # [k, c, oc]
    # tap-paired stationary: row r of pair j -> tap 2j + r//64, chan r%64
    wsm2 = np.zeros((128, 5, OC), np.float32)
    wsm2[:64, :4] = np.transpose(wT[0:8:2], (1, 0, 2))
    wsm2[64:, :4] = np.transpose(wT[1:8:2], (1, 0, 2))
    wsm2[:64, 4] = wT[8]
    wsm2 = np.ascontiguousarray(
        wsm2.reshape(128, 5 * OC).astype(bf16))

    kh = (np.arange(K2) // K).astype(np.float32)             # [9]
    kw = (np.arange(K2) % K).astype(np.float32)

    in_maps, meta = [], []
    for b in range(B):
        for h in range(2):
            lo = 0 if h == 0 else H - BAND
            out_lo = 0 if h == 0 else H - OUT_ROWS

            # --- quad table: entry e = [xT[e] | xT[e+160]] bf16 ---
            xb = x[b, :, lo:lo + BAND, :].reshape(CIN, N_TAB)
            xT = np.ascontiguousarray(xb.T).astype(bf16)     # [15360, 64]
            tab = np.zeros((N_TAB, 2 * CIN), bf16)
            tab[:N_TAB - W, :CIN] = xT[:N_TAB - W]
            tab[:N_TAB - W, CIN:] = xT[W:]

            # --- per-pixel sample positions (f32, mirrors reference) ---
            osl = offset[b, :, out_lo:out_lo + OUT_ROWS, :].reshape(18, NP)
            msl = mask[b, :, out_lo:out_lo + OUT_ROWS, :].reshape(K2, NP)
            rr = (out_lo + np.arange(OUT_ROWS, dtype=np.float32))[:, None] \
                .repeat(W, 1).reshape(NP)
            cc = np.arange(W, dtype=np.float32)[None, :] \
                .repeat(OUT_ROWS, 0).reshape(NP)
            py = (osl[0::2] + (rr[None, :] - 1.0 + kh[:, None])) \
                .astype(np.float32)                          # [9, NP]
            px = (osl[1::2] + (cc[None, :] - 1.0 + kw[:, None])) \
                .astype(np.float32)

            y0 = np.floor(py)
            x0 = np.floor(px)
            fy = py - y0
            fx = px - x0
            ly = y0 - np.float32(lo)
            ecy = np.clip(ly, 0.0, float(BAND - 2))
            tey = ly - ecy
            ecx = np.clip(x0, 0.0, float(W - 2))
            tex = x0 - ecx
            m = msl
            wyA = m * (np.where(tey == 0.0, 1.0 - fy, 0.0)
                       + np.where(tey == -1.0, fy, 0.0))
            wyB = m * (np.where(tey == 0.0, fy, 0.0)
                       + np.where(tey == 1.0, 1.0 - fy, 0.0))
            wxA = (np.where(tex == 0.0, 1.0 - fx, 0.0)
                   + np.where(tex == -1.0, fx, 0.0))
            wxB = (np.where(tex == 0.0, fx, 0.0)
                   + np.where(tex == 1.0, 1.0 - fx, 0.0))

            idx = (ecy * W + ecx).astype(np.int16)           # [9, NP]

            # wq[k, xj, yj, px] corner weights, mask folded into wy
            wq = np.empty((K2, 2, 2, NP), np.float32)
            wq[:, 0, 0] = wxA * wyA
            wq[:, 0, 1] = wxA * wyB
            wq[:, 1, 0] = wxB * wyA
            wq[:, 1, 1] = wxB * wyB

            # device wq2 layout: [128 p, c, (k,xj,yj), dup2] bf16
            wq_p = wq.reshape(K2, 4, NCHUNK, 128)            # px = c*128+p
            wq2 = np.transpose(wq_p, (3, 2, 0, 1))           # [p, c, k, j]
            wq2 = np.repeat(
                wq2.reshape(128, NCHUNK * 36), 2, axis=1).astype(bf16)

            # idx_wrap[p, (c*9+k)*8+dp] = idx[k, (c*128 + dp*16 + p%16)]
            idx_a = idx.reshape(K2, NCHUNK, 8, 16)           # [k, c, dp, q]
            wrap16 = np.transpose(idx_a, (3, 1, 0, 2))       # [q, c, k, dp]
            idx_wrap = np.tile(
                np.ascontiguousarray(wrap16.reshape(16, NIDX // 16)), (8, 1))

            in_maps.append({
                "tab": tab,
                "idxw": idx_wrap,
                "wq2": np.ascontiguousarray(wq2),
                "wsm2": wsm2,
            })
            meta.append((b, out_lo))
    return in_maps, meta


def kernel(x, offset, mask, weight):
    x = np.asarray(x, dtype=np.float32)
    offset = np.asarray(offset, dtype=np.float32)
    mask = np.asarray(mask, dtype=np.float32)
    weight = np.asarray(weight, dtype=np.float32)

    if "nc" not in _CACHE:
        _CACHE["nc"] = _build_program()
    nc = _CACHE["nc"]

    in_maps, meta = _host_inputs(x, offset, mask, weight)
    trace = os.environ.get("DEFORM_TRACE", "0") == "1"
    res = run_bass_kernel_spmd(nc, in_maps, core_ids=list(range(8)),
                               trace=trace)
    LAST_RESULTS["exec_time_ns"] = res.exec_time_ns
    LAST_RESULTS["mean_exec_time_ns"] = res.mean_exec_time_ns

    B = x.shape[0]
    out = np.zeros((B, OC, H, W), np.float32)
    for i, (b, out_lo) in enumerate(meta):
        out[b, :, out_lo:out_lo + OUT_ROWS, :] = \
            res.results[i]["out"].reshape(OC, OUT_ROWS, W)
    return out

